# revision 6
# baseline (speedup 1.0000x reference)
"""DenseMRConv (gnn message passing) on 8 TRN2 NeuronCores via Bass/Tile.

Math (reference):
    x_j  = x[edge_index]                      # [N, K, d] gather
    diff = max_k(x_j - x_i) = max_k(x_j) - x  # max distributes over const
    out  = concat([x, diff]) @ W + b
         = x @ (W_top - W_bot) + max_k(x_j) @ W_bot + b

Sharding: nodes (rows of x / edge_index) split across 8 cores; x is
replicated on every core as a bf16 gather table (edge_index addresses
global node ids); the small MLP weights are replicated.

Gather strategy: the HW gather primitive (InstDMAGatherAnt) takes int16
indices, so the table is laid out as 4 blocks of 32767 rows (+1 sentinel
row of -60000 per block).  Each edge is routed to its block with a local
int16 index.  Per 128-node tile and block, neighbor lists are padded to
the tile max count with sentinel indices (max() ignores the sentinel).
Rows are bf16 padded to a 256B stride (the gather's row-stride
granularity); payload per row is 128B.

Per tile: strided max-reduce over gathered slots -> M = max_k x_j, then
PE transposes + 3 accumulating matmuls:
  out = xT.T @ (W_top-W_bot) + MT.T @ W_bot + ones.T @ b
"""

import numpy as np

N, K, D, DOUT = 100000, 32, 64, 64
N_CORES = 8
P = 128
SHARD = N // N_CORES            # 12500 nodes per core
TILES = (SHARD + P - 1) // P    # 98
SHARD_PAD = TILES * P           # 12544
BLK = 32767                     # real rows per table block (int16 range)
NBLK = 4                        # ceil(100000 / 32767)
TROWS = NBLK * (BLK + 1)        # 131072 table rows incl. sentinels
SENT = -60000.0                 # sentinel value, below any data
GROUP = 4                       # tiles per gather group

TRACE = False                   # test.py sets True to collect HW exec time
LAST_EXEC_TIME_NS = None

_CACHE = {}


def _dma_gather_raw(gp, out_ap, in_ap, idxs_ap, num_idxs, elem_size, elem_step):
    """nc.gpsimd.dma_gather minus the 256B-payload assert (that restriction
    is transpose-only per the q7 ucode).  Payload may be any size; the row
    stride must be a multiple of 256B."""
    import concourse.mybir as mybir
    from concourse import ap_utils
    from concourse._compat import round_up_to_multiple
    from concourse.bass import MemorySpace

    assert idxs_ap.dtype == mybir.dt.int16
    assert in_ap.space == MemorySpace.DRAM
    assert idxs_ap.space == MemorySpace.SBUF and out_ap.space == MemorySpace.SBUF
    assert ap_utils.ap_is_contiguous(out_ap.ap[1:])
    assert ap_utils.ap_is_contiguous(idxs_ap.ap[1:])
    assert in_ap.ap[-1][1] == out_ap.ap[-1][1] == elem_size
    assert out_ap.ap[0][1] * out_ap.ap[1][1] == round_up_to_multiple(num_idxs, 128)
    assert in_ap.ap[0][0] == elem_step
    stride_bytes = elem_step * mybir.dt.size(in_ap.dtype)
    assert stride_bytes % 256 == 0 and stride_bytes // 256 < 256
    return gp.add_instruction(
        mybir.InstDMAGatherAnt(
            name=gp.bass.get_next_instruction_name(),
            ins=[*gp.lower_ap_dma(in_ap, for_custom_bir_dma=True),
                 gp.lower_ap(idxs_ap),
                 gp.lower_val_access(gp.to_reg(num_idxs))],
            outs=[gp.lower_ap(out_ap)],
            transpose=False,
            num_idxs=num_idxs,
            elem_size=elem_size,
            stride_bytes_256=stride_bytes // 256,
            gen_mode=0,
            single_packet=False,
            queue_num=0,
            sbuf_tokens_per_rank=0,
            sbuf_free_dim_per_rank=0,
            sbuf_free_dim_pad_per_rank=0,
            sbuf_byte_offset=0,
        ))


def _prep(x, edge_index, W, b):
    """Host-side sharding + gather-schedule construction.

    Returns (in_maps, meta).  meta (the gather schedule shapes) is
    identical for every core -- W widths are maxed across cores -- so one
    SPMD program serves all 8 cores; only the index DATA differs.
    """
    import ml_dtypes
    bf16 = ml_dtypes.bfloat16

    x = np.ascontiguousarray(np.asarray(x, dtype=np.float32))
    ei = np.asarray(edge_index).astype(np.int64)
    W_ = np.asarray(W, dtype=np.float32)
    b_ = np.asarray(b, dtype=np.float32).reshape(1, DOUT)
    A = np.ascontiguousarray(W_[:D] - W_[D:])
    Wb16 = np.ascontiguousarray(W_[D:].astype(bf16))

    # bf16 table: 4 blocks of (32767 real rows + 1 sentinel), 256B stride
    xa = np.zeros((TROWS, 2 * D), dtype=bf16)
    x16 = x.astype(bf16)
    for q in range(NBLK):
        lo = q * BLK
        hi = min(lo + BLK, N)
        xa[q * (BLK + 1): q * (BLK + 1) + (hi - lo), :D] = x16[lo:hi]
        xa[q * (BLK + 1) + BLK, :D] = bf16(SENT)

    # per-core quarter-sorted local indices + counts
    loc_all, cnt_all, start_all = [], [], []
    for c in range(N_CORES):
        eis = np.zeros((SHARD_PAD, K), np.int64)
        eis[:SHARD] = ei[c * SHARD:(c + 1) * SHARD]
        q = eis // BLK
        loc = eis % BLK
        order = np.argsort(q, axis=1, kind="stable")
        loc_s = np.take_along_axis(loc, order, 1).astype(np.int16)
        cnt = np.stack([(q == qq).sum(1) for qq in range(NBLK)], 1).astype(np.int32)
        cnt[SHARD:] = 0
        start = np.concatenate(
            [np.zeros((SHARD_PAD, 1), np.int32), np.cumsum(cnt, 1)[:, :NBLK - 1]], 1)
        loc_all.append(loc_s)
        cnt_all.append(cnt)
        start_all.append(start)

    # uniform widths: per (tile, block), max over cores and tile rows, >= 1
    cnt_t = np.stack(cnt_all).reshape(N_CORES, TILES, P, NBLK)
    Wt = np.maximum(1, cnt_t.max(axis=(0, 2)))          # [TILES, NBLK]

    groups = []
    t0 = 0
    while t0 < TILES:
        groups.append(list(range(t0, min(t0 + GROUP, TILES))))
        t0 += GROUP

    # build per-core wrapped int16 index stream + shared schedule meta
    meta = []           # per group: list of (q, S, coff) ; plus tiles & widths
    blocks_per_core = [[] for _ in range(N_CORES)]
    coff = 0
    for g in groups:
        entries = []
        for q in range(NBLK):
            S = int(Wt[g, q].sum())
            num = P * S
            for c in range(N_CORES):
                cols = []
                for t in g:
                    rows = slice(t * P, (t + 1) * P)
                    Wtq = int(Wt[t, q])
                    jj = np.arange(Wtq)[None, :]
                    valid = jj < cnt_all[c][rows, q, None]
                    srccol = np.minimum(start_all[c][rows, q, None] + jj, K - 1)
                    vals = np.where(valid,
                                    np.take_along_axis(loc_all[c][rows], srccol, 1),
                                    np.int16(BLK))
                    cols.append(vals.astype(np.int16))
                block = np.concatenate(cols, axis=1)          # [128, S]
                flat = block.T.reshape(-1)                    # n = s*128+p
                wrapped = np.tile(flat.reshape(-1, 16).T, (8, 1))
                blocks_per_core[c].append(wrapped)
            entries.append((q, S, coff))
            coff += num // 16
        meta.append({"tiles": g, "entries": entries,
                     "W": {q: [int(Wt[t, q]) for t in g] for q in range(NBLK)}})

    idx16 = [np.ascontiguousarray(np.concatenate(bs, axis=1))
             for bs in blocks_per_core]
    assert idx16[0].shape[1] == coff

    in_maps = []
    for c in range(N_CORES):
        lo = c * SHARD
        xs = np.zeros((SHARD_PAD, D), np.float32)
        xs[:SHARD] = x[lo:lo + SHARD]
        in_maps.append({
            "xa": xa, "xs": xs, "ix": idx16[c],
            "a": A, "wb": Wb16, "b": b_,
        })
    return in_maps, meta, coff


def _build(meta, idx_cols):
    import concourse.bacc as bacc
    import concourse.mybir as mybir
    import concourse.tile as tile
    from concourse.masks import make_identity

    f32 = mybir.dt.float32
    b16 = mybir.dt.bfloat16
    i16 = mybir.dt.int16

    nc = bacc.Bacc("TRN2", target_bir_lowering=False, debug=False,
                   num_devices=N_CORES)

    xa_d = nc.dram_tensor("xa", [TROWS, 2 * D], b16, kind="ExternalInput")
    xs_d = nc.dram_tensor("xs", [SHARD_PAD, D], f32, kind="ExternalInput")
    ix_d = nc.dram_tensor("ix", [P, idx_cols], i16, kind="ExternalInput")
    a_d = nc.dram_tensor("a", [D, DOUT], f32, kind="ExternalInput")
    wb_d = nc.dram_tensor("wb", [D, DOUT], b16, kind="ExternalInput")
    b_d = nc.dram_tensor("b", [1, DOUT], f32, kind="ExternalInput")
    out_d = nc.dram_tensor("out", [SHARD_PAD, DOUT], f32, kind="ExternalOutput")

    xs_t = xs_d.ap().rearrange("(t p) d -> t p d", p=P)
    out_t = out_d.ap().rearrange("(t p) d -> t p d", p=P)

    with tile.TileContext(nc) as tc:
        with (
            tc.tile_pool(name="const", bufs=1) as cpool,
            tc.tile_pool(name="gather", bufs=2) as gpool,
            tc.tile_pool(name="ixp", bufs=8) as ipool,
            tc.tile_pool(name="small", bufs=4) as spool,
            tc.tile_pool(name="mst", bufs=4) as mpool,
            tc.tile_pool(name="psum", bufs=2, space="PSUM") as ppool,
        ):
            identf = cpool.tile([P, P], f32)
            make_identity(nc, identf[:])
            identb = cpool.tile([P, P], b16)
            make_identity(nc, identb[:])
            ones1 = cpool.tile([1, P], f32)
            nc.gpsimd.memset(ones1[:], 1.0)
            a_t = cpool.tile([D, DOUT], f32)
            nc.sync.dma_start(a_t[:], a_d.ap())
            wb_t = cpool.tile([D, DOUT], b16)
            nc.sync.dma_start(wb_t[:], wb_d.ap())
            b_t = cpool.tile([1, DOUT], f32)
            nc.sync.dma_start(b_t[:], b_d.ap())

            for gmeta in meta:
                tiles = gmeta["tiles"]
                entries = gmeta["entries"]
                S_tot = sum(S for (_, S, _) in entries)
                gt = gpool.tile([P, S_tot * D], b16, tag="g")
                qoff = []
                off = 0
                for (q, S, coff) in entries:
                    qoff.append(off)
                    it = ipool.tile([P, 8 * S], i16, tag="ix")
                    nc.sync.dma_start(it[:], ix_d.ap()[:, coff: coff + 8 * S])
                    _dma_gather_raw(
                        nc.gpsimd,
                        out_ap=gt[:, off * D:(off + S) * D]
                            .rearrange("p (s e) -> p s e", e=D),
                        in_ap=xa_d.ap()[q * (BLK + 1):(q + 1) * (BLK + 1), :D],
                        idxs_ap=it[:],
                        num_idxs=P * S, elem_size=D, elem_step=2 * D)
                    off += S

                for ti, t in enumerate(tiles):
                    ms = mpool.tile([P, NBLK * D], b16, tag="ms")
                    for qi, (q, S, _) in enumerate(entries):
                        Wl = gmeta["W"][q]
                        toff = qoff[qi] + sum(Wl[:ti])
                        Wtq = Wl[ti]
                        view = gt[:, toff * D:(toff + Wtq) * D] \
                            .rearrange("p (s e) -> p e s", e=D)
                        nc.vector.tensor_reduce(
                            out=ms[:, qi * D:(qi + 1) * D], in_=view,
                            axis=mybir.AxisListType.X, op=mybir.AluOpType.max)
                    nc.vector.tensor_tensor(
                        out=ms[:, :2 * D], in0=ms[:, :2 * D], in1=ms[:, 2 * D:],
                        op=mybir.AluOpType.max)
                    nc.vector.tensor_tensor(
                        out=ms[:, :D], in0=ms[:, :D], in1=ms[:, D:2 * D],
                        op=mybir.AluOpType.max)

                    xt = spool.tile([P, D], f32, tag="xt")
                    nc.sync.dma_start(xt[:], xs_t[t])
                    xT_p = ppool.tile([D, P], f32, tag="xT", space="PSUM")
                    nc.tensor.transpose(out=xT_p[:], in_=xt[:], identity=identf[:])
                    mT_p = ppool.tile([D, P], b16, tag="mT", space="PSUM")
                    nc.tensor.transpose(out=mT_p[:], in_=ms[:, :D], identity=identb[:])
                    xT = spool.tile([D, P], f32, tag="xTs")
                    nc.scalar.copy(out=xT[:], in_=xT_p[:])
                    mT = spool.tile([D, P], b16, tag="mTs")
                    nc.scalar.copy(out=mT[:], in_=mT_p[:])

                    o_p = ppool.tile([P, DOUT], f32, tag="o", space="PSUM")
                    nc.tensor.matmul(o_p[:], lhsT=xT[:], rhs=a_t[:],
                                     start=True, stop=False)
                    nc.tensor.matmul(o_p[:], lhsT=mT[:], rhs=wb_t[:],
                                     start=False, stop=False)
                    nc.tensor.matmul(o_p[:], lhsT=ones1[:], rhs=b_t[:],
                                     start=False, stop=True)
                    o_s = spool.tile([P, DOUT], f32, tag="os")
                    nc.vector.tensor_copy(out=o_s[:], in_=o_p[:])
                    nc.sync.dma_start(out_t[t], o_s[:])

    nc.compile()
    return nc


def _install_trace_shim():
    """Provide antenv.axon_hooks (missing in this image) so
    run_bass_kernel_spmd(trace=True) can collect an NTFF profile."""
    import sys
    import types
    try:
        from antenv import axon_hooks  # noqa: F401
        return
    except ImportError:
        pass
    import antenv
    from concourse import bass_utils
    mod = types.ModuleType("antenv.axon_hooks")
    _hook = [None]
    mod.set_axon_ntff_profile_hook = lambda h: _hook.__setitem__(0, h)
    mod.get_axon_ntff_profile_hook = lambda: _hook[0]
    sys.modules["antenv.axon_hooks"] = mod
    antenv.axon_hooks = mod
    from trn_agent_boot.trn_boot import _ntff_profile_via_ctypes
    mod.set_axon_ntff_profile_hook(
        _ntff_profile_via_ctypes("/opt/axon/libaxon_pjrt.so"))
    bass_utils.upload_artifacts = lambda d: d


def kernel(x, edge_index, W, b):
    global LAST_EXEC_TIME_NS
    from concourse import bass_utils

    if TRACE:
        _install_trace_shim()

    in_maps, meta, idx_cols = _prep(x, edge_index, W, b)
    if "nc" not in _CACHE:
        _CACHE["nc"] = _build(meta, idx_cols)
    nc = _CACHE["nc"]

    res = bass_utils.run_bass_kernel_spmd(
        nc, in_maps, core_ids=list(range(N_CORES)), trace=TRACE,
    )
    LAST_EXEC_TIME_NS = res.exec_time_ns
    out = np.concatenate(
        [res.results[c]["out"][:SHARD] for c in range(N_CORES)], axis=0
    )
    return out.astype(np.float32)


if __name__ == "__main__":
    rng = np.random.default_rng(0)
    x = rng.standard_normal((N, D), dtype=np.float32)
    ei = rng.integers(0, N, (N, K)).astype(np.int64)
    W = (rng.standard_normal((2 * D, DOUT)) / np.sqrt(2 * D)).astype(np.float32)
    b = np.zeros(DOUT, np.float32)
    out = kernel(x, ei, W, b)
    M = np.max(x[ei], axis=1)
    exp = x @ (W[:D] - W[D:]) + M @ W[D:] + b
    err = np.abs(out - exp).max() / np.abs(exp).max()
    print("rel err:", err)
